# revision 20
# baseline (speedup 1.0000x reference)
"""Trainium2 Bass kernel for nn_LoRALayer: out = x @ W.T + b + 2.0*(x@A.T)@B.T.

Strategy: fold the LoRA update into the weight on the host —
out = x @ (W + 2*B@A).T + b exactly (associativity) — then run the
remaining dense GEMM 8-way data-parallel over tokens (1024/core). All
operand reshaping (transpose, bf16 cast, tile packing) happens on the
host, so the device program is a pure stream of back-to-back N=512 bf16
matmuls in out[t, o] orientation:

  - lhsT = xT k-tile [128i, 128t] (stationary), rhs = W2T k-tile
    [128i, 512o] (moving) accumulate psum [128t, 512o] over 32 k-tiles.
  - The bias is added during PSUM eviction (vector tensor_add against a
    partition-replicated bias tile), then DMA straight to out[t, o].
  - o-slice 0's eight psum groups are k-interleaved across all 8 PSUM
    banks so the PE streams at full rate while xt/wt tiles arrive.
"""

import os

import numpy as np

try:
    import concourse.bass as bass  # noqa: F401
except ImportError:  # pragma: no cover
    import sys

    sys.path.insert(0, "/opt/trn_rl_repo")
    import concourse.bass as bass  # noqa: F401

import concourse.tile as tile
from concourse import bacc, mybir
from concourse.bass_utils import run_bass_kernel_spmd

P = 128
N_CORES = 8
N_TOK = 8192
NT = N_TOK // N_CORES  # tokens per core (1024)
KD = 4096  # in_features (contraction)
OD = 4096  # out_features
R = 16
SCALING = 2.0

KT = KD // P  # 32 k-tiles
MT = NT // P  # 8 token tiles per core
OSL = 512  # out-feature slice width (one psum bank)
NOS = OD // OSL  # 8 out slices
WKK = KT // 2  # 16 wt tiles per slice (2 k-tiles packed per tile)

F32 = mybir.dt.float32
BF16 = mybir.dt.bfloat16
NP_BF16 = mybir.dt.np(BF16)

_NC_CACHE = None


def _build():
    from contextlib import ExitStack

    nc = bacc.Bacc("TRN2", target_bir_lowering=False, debug=False,
                   num_devices=N_CORES)
    # Host-prepped inputs: xt = x.T shard [4096i, 1024t] bf16;
    # wt = (W + 2BA).T packed [o, kk, 128p, 1024] -> [16384, 1024] bf16
    # where row (o*16+kk)*128+p holds [W2T[256kk+p, 512o:+512] |
    # W2T[256kk+128+p, 512o:+512]]; brep = bias replicated [128, 4096] f32.
    xt_d = nc.dram_tensor("xt", [KD, NT], BF16, kind="ExternalInput").ap()
    wt_d = nc.dram_tensor("wt", [NOS * WKK * P, 2 * OSL], BF16,
                          kind="ExternalInput").ap()
    brep_d = nc.dram_tensor("brep", [P, OD], F32, kind="ExternalInput").ap()
    out_d = nc.dram_tensor("out", [NT, OD], F32, kind="ExternalOutput").ap()

    with tile.TileContext(nc) as tc, ExitStack() as ctx:
        const = ctx.enter_context(tc.tile_pool(name="const", bufs=1))
        xt_pool = ctx.enter_context(tc.tile_pool(name="xt", bufs=KT))
        wt_pool = ctx.enter_context(tc.tile_pool(name="wt", bufs=3 * WKK))
        ob_pool = ctx.enter_context(tc.tile_pool(name="ob", bufs=4))
        ps_o = ctx.enter_context(tc.tile_pool(name="ps_o", bufs=8, space="PSUM"))

        # DMA issue order = PE consumption order: xt[k] and the wt0 tile
        # feeding the same k-step arrive together.
        xt = [None] * KT
        wt_tiles = {}

        def load_xt(k):
            t = xt_pool.tile([P, NT], BF16, tag="xt", name=f"xt{k}")
            nc.sync.dma_start(t[:], xt_d[k * P:(k + 1) * P, :])
            xt[k] = t

        def load_wt(o, kk):
            t = wt_pool.tile([P, 2 * OSL], BF16, tag="wt", name=f"wt{o}_{kk}")
            r0 = (o * WKK + kk) * P
            nc.sync.dma_start(t[:], wt_d[r0:r0 + P, :])
            wt_tiles[(o, kk)] = t

        def load_slab(o):
            for kk in range(WKK):
                load_wt(o, kk)

        for k in range(KT):
            load_xt(k)
            if k % 2 == 0:
                load_wt(0, k // 2)
        brep_sb = const.tile([P, OD], F32)
        nc.sync.dma_start(brep_sb[:], brep_d[:, :])
        load_slab(1)

        def main_mm(po, o, m, k):
            kk, j = divmod(k, 2)
            nc.tensor.matmul(po[:], xt[k][:, m * P:(m + 1) * P],
                             wt_tiles[(o, kk)][:, j * OSL:(j + 1) * OSL],
                             start=(k == 0), stop=(k == KT - 1))

        def evict_group(o, m, po, chunks=1):
            # chunks=2 on the very last group pipelines the bias-add with the
            # out-DMA, shortening the serialized post-last-matmul tail.
            ob = ob_pool.tile([P, OSL], F32, tag="ob", name=f"ob{o}_{m}")
            w = OSL // chunks
            for j in range(chunks):
                c0, c1 = o * OSL + j * w, o * OSL + (j + 1) * w
                nc.vector.tensor_add(ob[:, j * w:(j + 1) * w],
                                     po[:, j * w:(j + 1) * w],
                                     brep_sb[:, c0:c1])
                nc.sync.dma_start(
                    out_d[m * P:(m + 1) * P, c0:c1], ob[:, j * w:(j + 1) * w])

        # Warmup: ~24 garbage matmuls on a memset tile, dependent on nothing,
        # run during the ~8us DMA-ring init so HAM un-throttles the PE
        # (4/8 -> 8/8 takes ~3.4us of sustained activity) before real
        # operands arrive. Uses the first ps_o bank; real groups rotate on.
        wu = const.tile([P, OSL], BF16)
        nc.vector.memset(wu[:], 0.0)
        pw = ps_o.tile([P, OSL], F32, tag="po", name="pw")
        for _ in range(6):
            nc.tensor.matmul(pw[:], wu[:, 0:P], wu[:], start=True, stop=True)

        # Startup: all 8 groups of o-slice 0 k-interleaved across the 8
        # PSUM banks so the PE streams while xt/wt0 tiles are arriving.
        po0 = [ps_o.tile([P, OSL], F32, tag="po", name=f"po0_{m}")
               for m in range(MT)]
        for k in range(KT):
            for m in range(MT):
                main_mm(po0[m], 0, m, k)
        for m in range(MT):
            evict_group(0, m, po0[m])
        load_slab(2)

        # Steady state: one group per (o-slice, m), 32 K=128 matmuls each.
        for o in range(1, NOS):
            for m in range(MT):
                po = ps_o.tile([P, OSL], F32, tag="po", name=f"po{o}_{m}")
                for k in range(KT):
                    main_mm(po, o, m, k)
                last = (o == NOS - 1 and m == MT - 1)
                evict_group(o, m, po, chunks=2 if last else 1)
            # prefetch slab o+2 after all of slab o's readers are issued
            if o + 2 < NOS:
                load_slab(o + 2)

    nc.compile()
    return nc


def _get_nc():
    global _NC_CACHE
    if _NC_CACHE is None:
        _NC_CACHE = _build()
    return _NC_CACHE


def _prep_host(x, W, b, lora_A, lora_B):
    xb = np.asarray(x, dtype=np.float32).astype(NP_BF16)
    # Fold LoRA into the weight: out = x @ (W + 2*B@A).T + b exactly.
    W2 = np.asarray(W, dtype=np.float32) + SCALING * (
        np.asarray(lora_B, dtype=np.float32) @ np.asarray(lora_A, dtype=np.float32))
    # wt[(o*16+kk)*128+p, j*512+c] = W2.T[256kk+128j+p, 512o+c]
    #                              = W2[512o+c, 256kk+128j+p]
    Wb = W2.astype(NP_BF16)
    wt = np.ascontiguousarray(
        Wb.reshape(NOS, OSL, WKK, 2, P).transpose(0, 2, 4, 3, 1)
    ).reshape(NOS * WKK * P, 2 * OSL)
    brep = np.ascontiguousarray(
        np.broadcast_to(np.asarray(b, dtype=np.float32), (P, OD)))
    xts = [np.ascontiguousarray(xb[c * NT:(c + 1) * NT, :].T)
           for c in range(N_CORES)]
    return xts, wt, brep


def kernel(x, W, b, lora_A, lora_B):
    nc = _get_nc()
    xts, wt, brep = _prep_host(x, W, b, lora_A, lora_B)
    in_maps = [
        {"xt": xts[c], "wt": wt, "brep": brep}
        for c in range(N_CORES)
    ]
    res = run_bass_kernel_spmd(nc, in_maps, core_ids=list(range(N_CORES)),
                               trace=bool(int(os.environ.get("LORA_TRACE", "0"))))
    kernel.last_results = res
    return np.concatenate([res.results[c]["out"] for c in range(N_CORES)], axis=0)


if __name__ == "__main__":
    rng = np.random.default_rng(0)
    x = rng.standard_normal((N_TOK, KD), dtype=np.float32)
    W = (rng.standard_normal((OD, KD)) * 0.02).astype(np.float32)
    b = (rng.standard_normal(OD) * 0.02).astype(np.float32)
    A = (rng.standard_normal((R, KD)) * 0.02).astype(np.float32)
    B = (rng.standard_normal((OD, R)) * 0.02).astype(np.float32)
    out = kernel(x=x, W=W, b=b, lora_A=A, lora_B=B)
    ref = x.astype(np.float64) @ W.T.astype(np.float64) + b + SCALING * (
        (x.astype(np.float64) @ A.T.astype(np.float64)) @ B.T.astype(np.float64))
    rel = np.linalg.norm(out - ref) / np.linalg.norm(ref)
    print("rel_l2:", rel)


# revision 22
# speedup vs baseline: 1.0012x; 1.0012x over previous
"""Trainium2 Bass kernel for nn_LoRALayer: out = x @ W.T + b + 2.0*(x@A.T)@B.T.

Strategy: fold the LoRA update into the weight on the host —
out = x @ (W + 2*B@A).T + b exactly (associativity) — then run the
remaining dense GEMM 8-way data-parallel over tokens (1024/core). All
operand reshaping (transpose, bf16 cast, tile packing) happens on the
host, so the device program is a pure stream of back-to-back N=512 bf16
matmuls in out[t, o] orientation:

  - lhsT = xT k-tile [128i, 128t] (stationary), rhs = W2T k-tile
    [128i, 512o] (moving) accumulate psum [128t, 512o] over 32 k-tiles.
  - The bias is added during PSUM eviction (vector tensor_add against a
    partition-replicated bias tile), then DMA straight to out[t, o].
  - o-slice 0's eight psum groups are k-interleaved across all 8 PSUM
    banks so the PE streams at full rate while xt/wt tiles arrive.
"""

import os

import numpy as np

try:
    import concourse.bass as bass  # noqa: F401
except ImportError:  # pragma: no cover
    import sys

    sys.path.insert(0, "/opt/trn_rl_repo")
    import concourse.bass as bass  # noqa: F401

import concourse.tile as tile
from concourse import bacc, mybir
from concourse.bass_utils import run_bass_kernel_spmd

P = 128
N_CORES = 8
N_TOK = 8192
NT = N_TOK // N_CORES  # tokens per core (1024)
KD = 4096  # in_features (contraction)
OD = 4096  # out_features
R = 16
SCALING = 2.0

KT = KD // P  # 32 k-tiles
MT = NT // P  # 8 token tiles per core
OSL = 512  # out-feature slice width (one psum bank)
NOS = OD // OSL  # 8 out slices
WKK = KT // 2  # 16 wt tiles per slice (2 k-tiles packed per tile)

F32 = mybir.dt.float32
BF16 = mybir.dt.bfloat16
NP_BF16 = mybir.dt.np(BF16)

_NC_CACHE = None


def _build():
    from contextlib import ExitStack

    nc = bacc.Bacc("TRN2", target_bir_lowering=False, debug=False,
                   num_devices=N_CORES)
    # Host-prepped inputs: xt = x.T shard [4096i, 1024t] bf16;
    # wt = (W + 2BA).T packed [o, kk, 128p, 1024] -> [16384, 1024] bf16
    # where row (o*16+kk)*128+p holds [W2T[256kk+p, 512o:+512] |
    # W2T[256kk+128+p, 512o:+512]]; brep = bias replicated [128, 4096] f32.
    xt_d = nc.dram_tensor("xt", [KD, NT], BF16, kind="ExternalInput").ap()
    wt_d = nc.dram_tensor("wt", [NOS * WKK * P, 2 * OSL], BF16,
                          kind="ExternalInput").ap()
    brep_d = nc.dram_tensor("brep", [P, OD], F32, kind="ExternalInput").ap()
    out_d = nc.dram_tensor("out", [NT, OD], F32, kind="ExternalOutput").ap()

    with tile.TileContext(nc) as tc, ExitStack() as ctx:
        const = ctx.enter_context(tc.tile_pool(name="const", bufs=1))
        xt_pool = ctx.enter_context(tc.tile_pool(name="xt", bufs=KT))
        wt_pool = ctx.enter_context(tc.tile_pool(name="wt", bufs=3 * WKK))
        ob_pool = ctx.enter_context(tc.tile_pool(name="ob", bufs=4))
        ps_o = ctx.enter_context(tc.tile_pool(name="ps_o", bufs=8, space="PSUM"))

        # DMA issue order = PE consumption order: xt[k] and the wt0 tile
        # feeding the same k-step arrive together.
        xt = [None] * KT
        wt_tiles = {}

        def load_xt(k):
            t = xt_pool.tile([P, NT], BF16, tag="xt", name=f"xt{k}")
            nc.sync.dma_start(t[:], xt_d[k * P:(k + 1) * P, :])
            xt[k] = t

        def load_wt(o, kk):
            t = wt_pool.tile([P, 2 * OSL], BF16, tag="wt", name=f"wt{o}_{kk}")
            r0 = (o * WKK + kk) * P
            nc.sync.dma_start(t[:], wt_d[r0:r0 + P, :])
            wt_tiles[(o, kk)] = t

        def load_slab(o):
            for kk in range(WKK):
                load_wt(o, kk)

        for k in range(KT):
            load_xt(k)
            if k % 2 == 0:
                load_wt(0, k // 2)
        brep_sb = const.tile([P, OD], F32)
        nc.sync.dma_start(brep_sb[:], brep_d[:, :])
        load_slab(1)

        def main_mm(po, o, m, k):
            kk, j = divmod(k, 2)
            nc.tensor.matmul(po[:], xt[k][:, m * P:(m + 1) * P],
                             wt_tiles[(o, kk)][:, j * OSL:(j + 1) * OSL],
                             start=(k == 0), stop=(k == KT - 1))

        def evict_group(o, m, po):
            ob = ob_pool.tile([P, OSL], F32, tag="ob", name=f"ob{o}_{m}")
            nc.vector.tensor_add(ob[:], po[:],
                                 brep_sb[:, o * OSL:(o + 1) * OSL])
            nc.sync.dma_start(
                out_d[m * P:(m + 1) * P, o * OSL:(o + 1) * OSL], ob[:])

        # Warmup: ~24 garbage matmuls on a memset tile, dependent on nothing,
        # run during the ~8us DMA-ring init so HAM un-throttles the PE
        # (4/8 -> 8/8 takes ~3.4us of sustained activity) before real
        # operands arrive. Uses the first ps_o bank; real groups rotate on.
        wu = const.tile([P, OSL], BF16)
        nc.vector.memset(wu[:], 0.0)
        pw = ps_o.tile([P, OSL], F32, tag="po", name="pw")
        for _ in range(6):
            nc.tensor.matmul(pw[:], wu[:, 0:P], wu[:], start=True, stop=True)

        # Startup: all 8 groups of o-slice 0 k-interleaved across the 8
        # PSUM banks so the PE streams while xt/wt0 tiles are arriving.
        po0 = [ps_o.tile([P, OSL], F32, tag="po", name=f"po0_{m}")
               for m in range(MT)]
        for k in range(KT):
            for m in range(MT):
                main_mm(po0[m], 0, m, k)
        for m in range(MT):
            evict_group(0, m, po0[m])
        load_slab(2)

        # Steady state: one group per (o-slice, m), 32 K=128 matmuls each.
        for o in range(1, NOS):
            for m in range(MT):
                po = ps_o.tile([P, OSL], F32, tag="po", name=f"po{o}_{m}")
                for k in range(KT):
                    main_mm(po, o, m, k)
                evict_group(o, m, po)
            # prefetch slab o+2 after all of slab o's readers are issued
            if o + 2 < NOS:
                load_slab(o + 2)

    nc.compile()
    return nc


def _get_nc():
    global _NC_CACHE
    if _NC_CACHE is None:
        _NC_CACHE = _build()
    return _NC_CACHE


def _prep_host(x, W, b, lora_A, lora_B):
    xb = np.asarray(x, dtype=np.float32).astype(NP_BF16)
    # Fold LoRA into the weight: out = x @ (W + 2*B@A).T + b exactly.
    W2 = np.asarray(W, dtype=np.float32) + SCALING * (
        np.asarray(lora_B, dtype=np.float32) @ np.asarray(lora_A, dtype=np.float32))
    # wt[(o*16+kk)*128+p, j*512+c] = W2.T[256kk+128j+p, 512o+c]
    #                              = W2[512o+c, 256kk+128j+p]
    Wb = W2.astype(NP_BF16)
    wt = np.ascontiguousarray(
        Wb.reshape(NOS, OSL, WKK, 2, P).transpose(0, 2, 4, 3, 1)
    ).reshape(NOS * WKK * P, 2 * OSL)
    brep = np.ascontiguousarray(
        np.broadcast_to(np.asarray(b, dtype=np.float32), (P, OD)))
    xts = [np.ascontiguousarray(xb[c * NT:(c + 1) * NT, :].T)
           for c in range(N_CORES)]
    return xts, wt, brep


def kernel(x, W, b, lora_A, lora_B):
    nc = _get_nc()
    xts, wt, brep = _prep_host(x, W, b, lora_A, lora_B)
    in_maps = [
        {"xt": xts[c], "wt": wt, "brep": brep}
        for c in range(N_CORES)
    ]
    res = run_bass_kernel_spmd(nc, in_maps, core_ids=list(range(N_CORES)),
                               trace=bool(int(os.environ.get("LORA_TRACE", "0"))))
    kernel.last_results = res
    return np.concatenate([res.results[c]["out"] for c in range(N_CORES)], axis=0)


if __name__ == "__main__":
    rng = np.random.default_rng(0)
    x = rng.standard_normal((N_TOK, KD), dtype=np.float32)
    W = (rng.standard_normal((OD, KD)) * 0.02).astype(np.float32)
    b = (rng.standard_normal(OD) * 0.02).astype(np.float32)
    A = (rng.standard_normal((R, KD)) * 0.02).astype(np.float32)
    B = (rng.standard_normal((OD, R)) * 0.02).astype(np.float32)
    out = kernel(x=x, W=W, b=b, lora_A=A, lora_B=B)
    ref = x.astype(np.float64) @ W.T.astype(np.float64) + b + SCALING * (
        (x.astype(np.float64) @ A.T.astype(np.float64)) @ B.T.astype(np.float64))
    rel = np.linalg.norm(out - ref) / np.linalg.norm(ref)
    print("rel_l2:", rel)
